# revision 4
# baseline (speedup 1.0000x reference)
"""Trainium2 Bass kernel for nn_BoundingBoxDiscipline.

Computes PENALTY_WEIGHT * mean_B(area_penalty + center_offset) where the
penalties are derived from per-sample bounding boxes of thresholded masks:
    pred_mask = max_C(prediction_probs) > 0.3
    true_mask = max_C(expected_onehot)  > 0.5

The bounding box of a [H, W] mask only needs two tiny reductions:
    row_any[y] = any_{x,c} (v[y,x,c] > T)
    col_any[x] = any_{y,c} (v[y,x,c] > T)
so the device reduces 512 MiB of input down to 1 KiB of row/col summaries
per sample and the exact bbox/penalty math happens on the host (all
comparisons are exact: v > T <=> relu(v - T) > 0 in fp32, and sums of
non-negative values are > 0 iff any element is > 0).

Device plan, data-parallel over batch (2 samples per core, 8 cores):
  - DMA: 16 tiles of [128 y-rows, 8192 (x*16+c)] fp32 per core (64 MiB).
  - ScalarE: relu(v - T) -> bf16 tile, with accum_out giving the per-row
    (free-axis) sum in one pass -> row_any.
  - TensorE: ones[128,1].T @ relu_tile[:, :, c] accumulated over the 4
    y-tiles and 16 channel views into one PSUM [1, 512] -> col sums.
"""

import numpy as np

import concourse.bacc as bacc
import concourse.tile as tile
from concourse import mybir
from concourse.bass_utils import run_bass_kernel_spmd

N_CORES = 8
B, H, W, C = 16, 512, 512, 16
SAMPLES_PER_CORE = B // N_CORES          # 2
TILES_PER_SAMPLE = H // 128              # 4
FREE = W * C                             # 8192
THRESHOLDS = (0.3, 0.5)                  # (prediction_probs, expected_onehot)
PENALTY_WEIGHT = 0.05

f32 = mybir.dt.float32
bf16 = mybir.dt.bfloat16


def build_nc(repeat: int = 1):
    """Build the per-core Bass module. `repeat` wraps the body in a device
    loop (used only for wall-clock timing; the graded path uses repeat=1)."""
    nc = bacc.Bacc("TRN2", debug=False)

    n_tiles = 2 * SAMPLES_PER_CORE * TILES_PER_SAMPLE  # 16 (tensor, sample, ytile)
    n_st = 2 * SAMPLES_PER_CORE                        # 4 sample-tensors

    pred = nc.dram_tensor(
        "pred", [SAMPLES_PER_CORE * TILES_PER_SAMPLE, 128, FREE], f32,
        kind="ExternalInput").ap()
    exp = nc.dram_tensor(
        "exp", [SAMPLES_PER_CORE * TILES_PER_SAMPLE, 128, FREE], f32,
        kind="ExternalInput").ap()
    rows = nc.dram_tensor("rows", [128, n_tiles], f32, kind="ExternalOutput").ap()
    cols = nc.dram_tensor("cols", [n_st, W], f32, kind="ExternalOutput").ap()

    with tile.TileContext(nc) as tc:
        with (
            tc.tile_pool(name="singles", bufs=1) as singles,
            tc.tile_pool(name="loads", bufs=3) as loads,
            tc.tile_pool(name="relus", bufs=3) as relus,
            tc.tile_pool(name="rowsp", bufs=1) as rowsp,
            tc.tile_pool(name="colsb", bufs=2) as colsb,
            tc.tile_pool(name="psum", bufs=2, space="PSUM") as psum,
        ):
            ones = singles.tile([128, 1], bf16)
            nc.vector.memset(ones, 1.0)
            biases = []
            for thr in THRESHOLDS:
                bias_t = singles.tile([128, 1], f32, tag=f"bias{thr}")
                nc.vector.memset(bias_t, -thr)
                biases.append(bias_t)
            rows_sb = rowsp.tile([128, n_tiles], f32)

            def body(_iv=None):
                for tensor_idx, src in ((0, pred), (1, exp)):
                    bias_t = biases[tensor_idx]
                    for s in range(SAMPLES_PER_CORE):
                        st = tensor_idx * SAMPLES_PER_CORE + s
                        psum_t = psum.tile([1, W], f32)
                        for t in range(TILES_PER_SAMPLE):
                            k = tensor_idx * 8 + s * 4 + t
                            ld = loads.tile([128, FREE], f32)
                            nc.sync.dma_start(out=ld, in_=src[s * 4 + t])
                            rl = relus.tile([128, FREE], bf16)
                            nc.scalar.activation(
                                out=rl, in_=ld,
                                func=mybir.ActivationFunctionType.Relu,
                                bias=bias_t, scale=1.0,
                                accum_out=rows_sb[:, k : k + 1],
                            )
                            rl3 = rl.rearrange("p (x c) -> p x c", c=C)
                            for ci in range(C):
                                nc.tensor.matmul(
                                    psum_t, ones, rl3[:, :, ci],
                                    start=(t == 0 and ci == 0),
                                    stop=(t == TILES_PER_SAMPLE - 1 and ci == C - 1),
                                )
                        csb = colsb.tile([1, W], f32)
                        nc.vector.tensor_copy(csb, psum_t)
                        nc.sync.dma_start(out=cols[st : st + 1], in_=csb)
                nc.sync.dma_start(out=rows, in_=rows_sb)

            if repeat == 1:
                body()
            else:
                with tc.For_i(0, repeat, 1) as iv:
                    body(iv)

    nc.compile()
    return nc


def _shard_inputs(prediction_probs, expected_onehot):
    p = np.ascontiguousarray(np.asarray(prediction_probs), dtype=np.float32)
    e = np.ascontiguousarray(np.asarray(expected_onehot), dtype=np.float32)
    p = p.reshape(N_CORES, SAMPLES_PER_CORE * TILES_PER_SAMPLE, 128, FREE)
    e = e.reshape(N_CORES, SAMPLES_PER_CORE * TILES_PER_SAMPLE, 128, FREE)
    return [{"pred": p[c], "exp": e[c]} for c in range(N_CORES)]


def _bbox_from_any(row_any, col_any):
    ys = np.nonzero(row_any)[0]
    xs = np.nonzero(col_any)[0]
    if ys.size == 0:
        return 0, 0, 1, 1
    return int(ys[0]), int(xs[0]), int(ys[-1]), int(xs[-1])


def _combine(results):
    """Host epilogue: exact bbox/penalty math from row/col summaries."""
    f = np.float32
    penalties = []
    for core in range(N_CORES):
        rows = results[core]["rows"]  # [128, 16]
        cols = results[core]["cols"]  # [4, 512]
        for s in range(SAMPLES_PER_CORE):
            boxes = []
            for tensor_idx in range(2):
                k0 = tensor_idx * 8 + s * 4
                row_any = rows[:, k0 : k0 + 4].T.ravel() > 0  # y = t*128 + p
                col_any = cols[tensor_idx * SAMPLES_PER_CORE + s] > 0
                boxes.append(_bbox_from_any(row_any, col_any))
            (py1, px1, py2, px2), (ty1, tx1, ty2, tx2) = boxes
            pred_area = f((py2 - py1 + 1) * (px2 - px1 + 1))
            true_area = f((ty2 - ty1 + 1) * (tx2 - tx1 + 1))
            area_penalty = f(max(f(0.0), f(pred_area - true_area))) / f(true_area + f(1.0))
            pcy, pcx = f(py1 + py2) / f(2.0), f(px1 + px2) / f(2.0)
            tcy, tcx = f(ty1 + ty2) / f(2.0), f(tx1 + tx2) / f(2.0)
            center_offset = np.sqrt(np.square(f(pcy - tcy)) + np.square(f(pcx - tcx))) / f(20.0)
            penalties.append(f(area_penalty + center_offset))
    mean = np.mean(np.asarray(penalties, dtype=np.float32), dtype=np.float32)
    return np.asarray(np.float32(PENALTY_WEIGHT) * mean, dtype=np.float32)


_NC_CACHE = {}


def kernel(prediction_probs, expected_onehot):
    if "nc" not in _NC_CACHE:
        _NC_CACHE["nc"] = build_nc()
    nc = _NC_CACHE["nc"]
    in_maps = _shard_inputs(prediction_probs, expected_onehot)
    res = run_bass_kernel_spmd(nc, in_maps, core_ids=list(range(N_CORES)))
    return _combine(res.results)
